# revision 1
# baseline (speedup 1.0000x reference)
"""Trainium2 Bass kernel for GQA multi-head attention (nn_MultiHeadAttention).

Reference computation (fp32):
    q = h @ Wq^T -> RoPE ; k = h @ Wk^T -> RoPE ; v = h @ Wv^T
    scores = q k^T / sqrt(64) + causal_mask ; w = softmax(scores)
    out = (w v) @ Wo^T

Shapes: h [2,2048,2048], Wq [2048,2048], Wk/Wv [512,2048], Wo [2048,2048],
32 q heads / 8 kv heads (GQA group=4), head_dim 64.

Sharding: tensor-parallel over the 8 kv-head groups, one group per core.
Core g owns q heads [4g,4g+4), kv head g, Wo columns [256g, 256(g+1)).
Each core computes a full-token partial of the output projection; the host
sums the 8 partials (the Wo contraction splits over head blocks).

Per-core kernel layout trick: everything is kept transposed.  The host
passes h^T [2048, 4096(=b*s)], so the QKV projections produce Q^T/K^T
[head_dim, t] directly (lhsT = W^T block, rhs = h^T block).  Scores are
computed transposed, S^T[k, q] = (K^T)^T-free x Q^T, softmax runs as
exp(S^T) (no max subtraction -- scores are O(5) so exp is safe in fp32)
with causal blocks skipped and diagonal straddles masked multiplicatively
after exp.  A@V uses V augmented with a ones-column so the softmax
denominators fall out of the same matmul chain.  Final projection
out^T = Wo_g^T-block^T x attn^T needs no transposes anywhere on-device
except V ([d,t] -> [t,d]) which goes through the PE transpose path.
"""

import sys

for _p in ("/opt/trn_rl_repo",):
    if _p not in sys.path:
        sys.path.insert(0, _p)

import numpy as np
import ml_dtypes

D = 2048          # model dim
HD = 64           # head dim
S = 2048          # sequence
B = 2             # batch
T = B * S         # total tokens
EQ = 256          # q-projection rows per core (4 heads x 64)
TT = 512          # token tile for projections
NT = T // TT
NDB = D // 128    # contraction blocks for projections
QT = 512          # query tile for attention
KBLK = 128        # key block for attention
NQT = S // QT     # query tiles per batch
BF16 = ml_dtypes.bfloat16

_CACHE = {}


def _build_program(causal: bool):
    """Build the single-core Bass/Tile program (identical across cores)."""
    import concourse.bass as bass
    import concourse.mybir as mybir
    import concourse.tile as tile
    from concourse import bacc
    from concourse.masks import make_identity

    f32 = mybir.dt.float32
    bf16 = mybir.dt.bfloat16

    nc = bacc.Bacc("TRN2", target_bir_lowering=False, debug=False)

    hT = nc.dram_tensor("hT", [D, T], bf16, kind="ExternalInput").ap()
    wqT = nc.dram_tensor("wqT", [D, EQ], bf16, kind="ExternalInput").ap()
    # k and v projection weights packed [D, 64+64] so one matmul produces both
    wkvT = nc.dram_tensor("wkvT", [D, 2 * HD], bf16, kind="ExternalInput").ap()
    woT = nc.dram_tensor("woT", [EQ, D], bf16, kind="ExternalInput").ap()
    cos2 = nc.dram_tensor("cos2", [128, T], f32, kind="ExternalInput").ap()
    sin2s = nc.dram_tensor("sin2s", [128, T], f32, kind="ExternalInput").ap()
    # mask^T tiles, only used on the straddle diagonal when causal=False
    maskT = nc.dram_tensor("maskT", [S, S], f32, kind="ExternalInput").ap()
    outT = nc.dram_tensor("outT", [D, T], f32, kind="ExternalOutput").ap()

    hT_b = hT.rearrange("(n p) t -> n p t", p=128)
    wqT_b = wqT.rearrange("(n p) e -> p n e", p=128)
    wkvT_b = wkvT.rearrange("(n p) e -> p n e", p=128)
    woT_b = woT.rearrange("(n p) e -> p n e", p=128)
    outT_b = outT.rearrange("(n p) t -> n p t", p=128)

    Exp = mybir.ActivationFunctionType.Exp
    PSUM = bass.MemorySpace.PSUM

    with tile.TileContext(nc) as tc:
        import contextlib

        with contextlib.ExitStack() as stack:
            const = stack.enter_context(tc.tile_pool(name="const", bufs=1))

            wq_s = const.tile([128, NDB, EQ], bf16)
            wkv_s = const.tile([128, NDB, 2 * HD], bf16)
            wo_s = const.tile([128, 2, D], bf16)
            cos_s = const.tile([128, T], f32)
            sin_s = const.tile([128, T], f32)
            qt_s = [
                const.tile([128, T], bf16, tag=f"qt{i}", name=f"qt{i}")
                for i in range(2)
            ]
            kt_s = const.tile([128, T], bf16)
            va_s = const.tile([128, T // 128, HD + 1], bf16)
            tri_s = const.tile([128, 4, QT], bf16)
            ident = const.tile([128, 128], f32)

            nc.sync.dma_start(out=wq_s, in_=wqT_b)
            nc.sync.dma_start(out=wkv_s, in_=wkvT_b)
            nc.sync.dma_start(out=wo_s, in_=woT_b)
            nc.sync.dma_start(out=cos_s, in_=cos2)
            nc.sync.dma_start(out=sin_s, in_=sin2s)

            make_identity(nc, ident)
            # ones column of the augmented V
            nc.gpsimd.memset(va_s[:, :, HD : HD + 1], 1.0)
            # multiplicative causal masks for the 4 straddle offsets:
            # tri_s[p, j, f] = 1.0 where f >= p + 128*j else 0.0
            for j in range(4):
                nc.gpsimd.memset(tri_s[:, j, :], 1.0)
                nc.gpsimd.affine_select(
                    out=tri_s[:, j, :],
                    in_=tri_s[:, j, :],
                    compare_op=mybir.AluOpType.is_ge,
                    fill=0.0,
                    base=-128 * j,
                    channel_multiplier=-1,
                    pattern=[[1, QT]],
                )

            # ---------------- Phase A: QKV projections + RoPE + V transpose
            with contextlib.ExitStack() as pa:
                ht_pool = pa.enter_context(tc.tile_pool(name="ht", bufs=4))
                sc_pool = pa.enter_context(tc.tile_pool(name="pa_sc", bufs=2))
                ps_proj = pa.enter_context(
                    tc.tile_pool(name="pa_ps", bufs=2, space=PSUM)
                )
                ps_vt = pa.enter_context(
                    tc.tile_pool(name="pa_vt", bufs=2, space=PSUM)
                )

                for it in range(NT):
                    t0 = it * TT
                    tsl = slice(t0, t0 + TT)
                    q01 = ps_proj.tile([128, TT], f32, tag="q01")
                    q23 = ps_proj.tile([128, TT], f32, tag="q23")
                    kv = ps_proj.tile([128, TT], f32, tag="kv")
                    for idb in range(NDB):
                        htile = ht_pool.tile([128, TT], bf16)
                        nc.sync.dma_start(out=htile, in_=hT_b[idb, :, tsl])
                        first, last = idb == 0, idb == NDB - 1
                        nc.tensor.matmul(
                            q01, wq_s[:, idb, 0:128], htile, start=first, stop=last
                        )
                        nc.tensor.matmul(
                            q23, wq_s[:, idb, 128:256], htile, start=first, stop=last
                        )
                        nc.tensor.matmul(
                            kv, wkv_s[:, idb, :], htile, start=first, stop=last
                        )

                    # RoPE on the two stacked q head-pairs and on k.
                    # out = x*cos + rot_half(x)*sin_signed.  rot_half is a
                    # partition swap, which engines can't do, and DMA can't
                    # read PSUM -- so first copy PSUM->SBUF (same partitions),
                    # then partition-swap with SBUF->SBUF DMAs.
                    def rope(src_ap, nrows, dst_ap):
                        xf = sc_pool.tile([128, TT], f32, tag="xf")
                        m1 = sc_pool.tile([128, TT], f32, tag="m1")
                        m2 = sc_pool.tile([128, TT], f32, tag="m2")
                        tmp = sc_pool.tile([128, TT], f32, tag="tmp")
                        # ACT is idle during the projection phase; put the
                        # PSUM->SBUF staging copy there instead of on DVE
                        nc.scalar.copy(out=xf[:nrows], in_=src_ap[:nrows])
                        for c in range(nrows // 32):
                            lo = (c // 2) * 64 + (32 if c % 2 == 0 else 0)
                            nc.sync.dma_start(
                                out=tmp[c * 32 : c * 32 + 32, :],
                                in_=xf[lo : lo + 32, :],
                            )
                        nc.vector.tensor_mul(
                            m1[:nrows], src_ap[:nrows], cos_s[:nrows, tsl]
                        )
                        nc.vector.tensor_mul(
                            m2[:nrows], tmp[:nrows], sin_s[:nrows, tsl]
                        )
                        nc.vector.tensor_add(dst_ap, m1[:nrows], m2[:nrows])

                    rope(q01, 128, qt_s[0][:, tsl])
                    rope(q23, 128, qt_s[1][:, tsl])
                    rope(kv, 64, kt_s[0:64, tsl])
                    # replicate k rows so odd q-heads can matmul from
                    # partition base 64 (tile_position row packing)
                    nc.sync.dma_start(out=kt_s[64:128, tsl], in_=kt_s[0:64, tsl])

                    # V: [d, t] -> [t, d] through PE transpose.  V sits at
                    # partitions 64:128 of kv; keep it there (same-base copy)
                    # and transpose from base 64 with the bottom-right
                    # identity block.
                    v_sb = sc_pool.tile([128, TT], f32, tag="v_sb")
                    nc.scalar.copy(out=v_sb[64:128, :], in_=kv[64:128, :])
                    for c4 in range(TT // 128):
                        vt_ps = ps_vt.tile([128, HD], f32, tag="vt")
                        nc.tensor.transpose(
                            vt_ps,
                            v_sb[64:128, c4 * 128 : (c4 + 1) * 128],
                            ident[64:128, 64:128],
                        )
                        nc.vector.tensor_copy(
                            out=va_s[:, it * 4 + c4, 0:HD], in_=vt_ps
                        )

            # ---------------- Phase B: attention + output projection
            with contextlib.ExitStack() as pb:
                ps_s = pb.enter_context(tc.tile_pool(name="pb_s", bufs=2, space=PSUM))
                ps_o = pb.enter_context(tc.tile_pool(name="pb_o", bufs=2, space=PSUM))
                ps_w = pb.enter_context(tc.tile_pool(name="pb_w", bufs=2, space=PSUM))
                pt_pool = pb.enter_context(tc.tile_pool(name="pt", bufs=4))
                on_pool = pb.enter_context(tc.tile_pool(name="on", bufs=2))
                nm_pool = pb.enter_context(tc.tile_pool(name="nm", bufs=2))

                for b in range(B):
                    for iq in range(NQT):
                        q0 = iq * QT
                        qsl = slice(b * S + q0, b * S + q0 + QT)
                        on_t = [
                            on_pool.tile(
                                [128, QT], bf16, tag=f"on{i}", name=f"on{i}"
                            )
                            for i in range(2)
                        ]
                        for rp in range(2):
                            # head-pair (2rp, 2rp+1): the two K=64 S matmuls
                            # go to PE row-groups 0 and 64 (kt_s replication +
                            # matching qtile bases) so they run concurrently
                            # in the array, and one exp covers both heads.
                            qtile = qt_s[rp]
                            nkb = (q0 // KBLK + 4) if causal else (S // KBLK)
                            o_ps = [
                                ps_o.tile(
                                    [65, QT], f32, tag=f"o{i}", name=f"o{i}",
                                    bufs=1,
                                )
                                for i in range(2)
                            ]
                            for kb in range(nkb):
                                ksl = slice(
                                    b * S + kb * KBLK, b * S + (kb + 1) * KBLK
                                )
                                s_ps = ps_s.tile([128, 2, QT], f32, tag="s")
                                pt = pt_pool.tile([128, 2, QT], bf16, tag="pt")
                                for h in range(2):
                                    hb = h * 64
                                    nc.tensor.matmul(
                                        s_ps[:, h, :],
                                        kt_s[hb : hb + 64, ksl],
                                        qtile[hb : hb + 64, qsl],
                                        start=True,
                                        stop=True,
                                    )
                                if causal:
                                    nc.scalar.activation(
                                        pt, s_ps, Exp, scale=0.125
                                    )
                                else:
                                    mk = pt_pool.tile([128, QT], f32, tag="mk")
                                    sm = pt_pool.tile(
                                        [128, 2, QT], f32, tag="sm"
                                    )
                                    nc.sync.dma_start(
                                        out=mk,
                                        in_=maskT[
                                            kb * KBLK : (kb + 1) * KBLK,
                                            q0 : q0 + QT,
                                        ],
                                    )
                                    for h in range(2):
                                        nc.vector.scalar_tensor_tensor(
                                            out=sm[:, h, :],
                                            in0=s_ps[:, h, :],
                                            scalar=0.125,
                                            in1=mk,
                                            op0=mybir.AluOpType.mult,
                                            op1=mybir.AluOpType.add,
                                        )
                                    nc.scalar.activation(pt, sm, Exp, scale=1.0)
                                j = kb - q0 // KBLK
                                for h in range(2):
                                    if causal and j >= 0:
                                        nc.vector.tensor_mul(
                                            pt[:, h, :], pt[:, h, :],
                                            tri_s[:, j, :],
                                        )
                                    nc.tensor.matmul(
                                        o_ps[h],
                                        va_s[:, b * (S // 128) + kb, :],
                                        pt[:, h, :],
                                        start=(kb == 0),
                                        stop=(kb == nkb - 1),
                                    )
                            for h in range(2):
                                # normalize: row 64 of o_ps holds the softmax
                                # sums.  One copy PSUM->SBUF releases o_ps
                                # early; reciprocal of a 1-partition row runs
                                # on a single DVE lane (~3.3us), so bounce it
                                # through a [32, 16] layout via DMA to use 32
                                # lanes.
                                ou = nm_pool.tile([65, QT], f32, tag="ou")
                                nc.vector.tensor_copy(out=ou, in_=o_ps[h])
                                r32 = nm_pool.tile([32, 16], f32, tag="r32")
                                nc.sync.dma_start(out=r32, in_=ou[64:65, :])
                                r32r = nm_pool.tile([32, 16], f32, tag="r32r")
                                nc.vector.reciprocal(r32r, r32)
                                rec = nm_pool.tile([1, QT], f32, tag="rc")
                                nc.sync.dma_start(out=rec, in_=r32r)
                                rec_b = nm_pool.tile([64, QT], f32, tag="rb")
                                nc.gpsimd.partition_broadcast(rec_b, rec)
                                # engines can write shifted partition bases
                                # (verified on hw): odd heads write rows
                                # 64:128 directly
                                nc.vector.tensor_mul(
                                    on_t[rp][h * 64 : h * 64 + 64, :],
                                    ou[0:64, :],
                                    rec_b,
                                )
                        # output projection for this query tile
                        for eb in range(D // 128):
                            wo_ps = ps_w.tile([128, QT], f32, tag="wo")
                            for db in range(2):
                                nc.tensor.matmul(
                                    wo_ps,
                                    wo_s[:, db, eb * 128 : (eb + 1) * 128],
                                    on_t[db],
                                    start=(db == 0),
                                    stop=(db == 1),
                                )
                            wo_sb = pt_pool.tile([128, QT], f32, tag="wo_sb")
                            nc.vector.tensor_copy(out=wo_sb, in_=wo_ps)
                            nc.sync.dma_start(out=outT_b[eb, :, qsl], in_=wo_sb)

    nc.compile()
    return nc


def _host_inputs(inputs, causal):
    """Shard + transpose the full inputs into 8 per-core input maps."""
    h = np.asarray(inputs["hidden_states"], np.float32)
    cos = np.asarray(inputs["position_cos"], np.float32)
    sin = np.asarray(inputs["position_sin"], np.float32)
    Wq = np.asarray(inputs["Wq"], np.float32)
    Wk = np.asarray(inputs["Wk"], np.float32)
    Wv = np.asarray(inputs["Wv"], np.float32)
    Wo = np.asarray(inputs["Wo"], np.float32)
    mask = np.asarray(inputs["attention_mask"], np.float32)[0, 0]

    hT = np.ascontiguousarray(h.reshape(T, D).T).astype(BF16)

    cosT = np.tile(cos.T, (1, B))                     # [64, T]
    sinT = np.tile(sin.T, (1, B))
    cos2 = np.ascontiguousarray(np.vstack([cosT, cosT]).astype(np.float32))
    s_signed = np.vstack([-sinT[0:32], sinT[32:64]])  # rot_half sign baked in
    sin2s = np.ascontiguousarray(np.vstack([s_signed, s_signed]).astype(np.float32))

    maskT = np.ascontiguousarray(mask.T).astype(np.float32)

    in_maps = []
    for g in range(8):
        in_maps.append(
            {
                "hT": hT,
                "wqT": np.ascontiguousarray(
                    Wq[g * EQ : (g + 1) * EQ].T
                ).astype(BF16),
                "wkvT": np.ascontiguousarray(
                    np.concatenate(
                        [
                            Wk[g * HD : (g + 1) * HD].T,
                            Wv[g * HD : (g + 1) * HD].T,
                        ],
                        axis=1,
                    )
                ).astype(BF16),
                "woT": np.ascontiguousarray(
                    Wo[:, g * EQ : (g + 1) * EQ].T
                ).astype(BF16),
                "cos2": cos2,
                "sin2s": sin2s,
                "maskT": maskT,
            }
        )
    return in_maps


def _is_causal(mask):
    m = np.asarray(mask, np.float32)[0, 0]
    tri = np.tril(np.ones((S, S), bool))
    return bool(np.all(m[tri] == 0.0) and np.all(m[~tri] <= -1e8))


def _assemble(results):
    acc = np.zeros((D, T), np.float64)
    for r in results:
        acc += r["outT"].astype(np.float64)
    outT = acc.astype(np.float32)
    return np.ascontiguousarray(outT.reshape(D, B, S).transpose(1, 2, 0))


def kernel(**inputs) -> np.ndarray:
    from concourse.bass_utils import run_bass_kernel_spmd

    causal = _is_causal(inputs["attention_mask"])
    key = ("prog", causal)
    if key not in _CACHE:
        _CACHE[key] = _build_program(causal)
    nc = _CACHE[key]

    in_maps = _host_inputs(inputs, causal)
    res = run_bass_kernel_spmd(nc, in_maps, core_ids=list(range(8)))
    return _assemble(res.results)



# revision 2
# speedup vs baseline: 1.0172x; 1.0172x over previous
"""Trainium2 Bass kernel for GQA multi-head attention (nn_MultiHeadAttention).

Reference computation (fp32):
    q = h @ Wq^T -> RoPE ; k = h @ Wk^T -> RoPE ; v = h @ Wv^T
    scores = q k^T / sqrt(64) + causal_mask ; w = softmax(scores)
    out = (w v) @ Wo^T

Shapes: h [2,2048,2048], Wq [2048,2048], Wk/Wv [512,2048], Wo [2048,2048],
32 q heads / 8 kv heads (GQA group=4), head_dim 64.

Sharding: tensor-parallel over the 8 kv-head groups, one group per core.
Core g owns q heads [4g,4g+4), kv head g, Wo columns [256g, 256(g+1)).
Each core computes a full-token partial of the output projection; the host
sums the 8 partials (the Wo contraction splits over head blocks).

v2 structure (single software-pipelined loop over 8 token tiles of 512):
    step s:  [hT dma(s)] [attention for query tile s-1] [QKV proj + RoPE
              for tile s] [Wo projection + output store for tile s-1]
so every engine (PE / ACT / DVE / GPSIMD / DMA) has dense interleaved work
and the PE never idles long enough for the HAM clock gate to re-throttle.

Everything is kept transposed: h^T [2048, 4096] comes in, Q^T/K^T [d, t]
fall out of the projections directly, scores are S^T[k, q], softmax is a
plain exp (scores are O(5), fp32-safe) with causal-skip at 128-key-block
granularity AND 128-query-column truncation inside the diagonal straddle
blocks (scores / exp / mask / A@V all skip the dead triangle).  A@V uses V
augmented with a ones-column so softmax denominators fall out of the same
matmul.  RoPE's rot_half partition swap is done on a pre-scaled copy
(z = x * sin_pre, then swap z) so no PSUM->SBUF staging copy is needed on
the scalar engine.  Denominator reciprocals for all 4 heads of a query
tile are batched through one [32, 64] bounce so the DVE uses 32 lanes.
Output partials are stored bf16 (halves the HBM write) as one coalesced
DMA per query tile.
"""

import sys

for _p in ("/opt/trn_rl_repo",):
    if _p not in sys.path:
        sys.path.insert(0, _p)

import numpy as np
import ml_dtypes

D = 2048          # model dim
HD = 64           # head dim
S = 2048          # sequence
B = 2             # batch
T = B * S         # total tokens
EQ = 256          # q-projection rows per core (4 heads x 64)
TT = 512          # token tile (both projection and query tile)
NT = T // TT      # 8 merged steps
NDB = D // 128    # contraction blocks for projections
QT = 512          # query tile for attention
KBLK = 128        # key block for attention
BF16 = ml_dtypes.bfloat16

_CACHE = {}


def _build_program(causal: bool):
    """Build the single-core Bass/Tile program (identical across cores)."""
    import concourse.bass as bass
    import concourse.mybir as mybir
    import concourse.tile as tile
    from concourse import bacc
    from concourse.masks import make_identity

    f32 = mybir.dt.float32
    bf16 = mybir.dt.bfloat16

    nc = bacc.Bacc("TRN2", target_bir_lowering=False, debug=False)

    hT = nc.dram_tensor("hT", [D, T], bf16, kind="ExternalInput").ap()
    wqT = nc.dram_tensor("wqT", [D, EQ], bf16, kind="ExternalInput").ap()
    # k and v projection weights packed [D, 64+64] so one matmul produces both
    wkvT = nc.dram_tensor("wkvT", [D, 2 * HD], bf16, kind="ExternalInput").ap()
    woT = nc.dram_tensor("woT", [EQ, D], bf16, kind="ExternalInput").ap()
    cos2 = nc.dram_tensor("cos2", [128, T], f32, kind="ExternalInput").ap()
    # sin with rot_half sign AND partition swap pre-applied (see _host_inputs)
    sinp = nc.dram_tensor("sinp", [128, T], f32, kind="ExternalInput").ap()
    # mask^T tiles, only used on the straddle diagonal when causal=False
    maskT = nc.dram_tensor("maskT", [S, S], f32, kind="ExternalInput").ap()
    outT = nc.dram_tensor("outT", [D, T], bf16, kind="ExternalOutput").ap()

    hT_b3 = hT.rearrange("(n p) t -> p n t", p=128)     # [128, 16, T]
    wqT_b = wqT.rearrange("(n p) e -> p n e", p=128)
    wkvT_b = wkvT.rearrange("(n p) e -> p n e", p=128)
    woT_b = woT.rearrange("(n p) e -> p n e", p=128)
    outT_b3 = outT.rearrange("(n p) t -> p n t", p=128)  # [128, 16, T]

    Exp = mybir.ActivationFunctionType.Exp
    PSUM = bass.MemorySpace.PSUM

    with tile.TileContext(nc) as tc:
        import contextlib

        with contextlib.ExitStack() as stack:
            const = stack.enter_context(tc.tile_pool(name="const", bufs=1))

            wq_s = const.tile([128, NDB, EQ], bf16)
            wkv_s = const.tile([128, NDB, 2 * HD], bf16)
            wo_s = const.tile([128, 2, D], bf16)
            cos_s = const.tile([128, T], f32)
            sinp_s = const.tile([128, T], f32)
            qt_s = [
                const.tile([128, T], bf16, tag=f"qt{i}", name=f"qt{i}")
                for i in range(2)
            ]
            kt_s = const.tile([128, T], bf16)
            va_s = const.tile([128, T // 128, HD + 1], bf16)
            tri_s = const.tile([128, 4, QT], bf16)
            ident = const.tile([128, 128], f32)

            nc.sync.dma_start(out=wq_s, in_=wqT_b)
            nc.sync.dma_start(out=wkv_s, in_=wkvT_b)
            nc.sync.dma_start(out=wo_s, in_=woT_b)
            nc.sync.dma_start(out=cos_s, in_=cos2)
            nc.sync.dma_start(out=sinp_s, in_=sinp)

            make_identity(nc, ident)
            # ones column of the augmented V
            nc.gpsimd.memset(va_s[:, :, HD : HD + 1], 1.0)
            # multiplicative causal masks for the 4 straddle offsets:
            # tri_s[p, j, f] = 1.0 where f >= p + 128*j else 0.0
            for j in range(4):
                nc.gpsimd.memset(tri_s[:, j, :], 1.0)
                nc.gpsimd.affine_select(
                    out=tri_s[:, j, :],
                    in_=tri_s[:, j, :],
                    compare_op=mybir.AluOpType.is_ge,
                    fill=0.0,
                    base=-128 * j,
                    channel_multiplier=-1,
                    pattern=[[1, QT]],
                )

            # ---------------- pools for the merged pipeline
            with contextlib.ExitStack() as pp:
                ht_pool = pp.enter_context(tc.tile_pool(name="ht", bufs=2))
                # m1 / z / swapped-z rope scratch, all three ropes stacked
                rp_pool = pp.enter_context(tc.tile_pool(name="rp", bufs=1))
                vs_pool = pp.enter_context(tc.tile_pool(name="vs", bufs=2))
                # shared-PSUM pool: proj accumulators, V transposes, Wo tiles
                ps_mm = pp.enter_context(
                    tc.tile_pool(name="ps_mm", bufs=2, space=PSUM)
                )
                ps_s = pp.enter_context(
                    tc.tile_pool(name="ps_s", bufs=2, space=PSUM)
                )
                ps_o = pp.enter_context(
                    tc.tile_pool(name="ps_o", bufs=1, space=PSUM)
                )
                pt_pool = pp.enter_context(tc.tile_pool(name="pt", bufs=4))
                on_pool = pp.enter_context(tc.tile_pool(name="on", bufs=2))
                nm_pool = pp.enter_context(tc.tile_pool(name="nm", bufs=1))
                oa_pool = pp.enter_context(tc.tile_pool(name="oa", bufs=1))

                def proj(it):
                    """QKV projection + RoPE + V transpose for token tile it."""
                    t0 = it * TT
                    tsl = slice(t0, t0 + TT)
                    htile = ht_pool.tile([128, NDB, TT], bf16, tag="ht")
                    nc.sync.dma_start(out=htile, in_=hT_b3[:, :, tsl])

                    m1_all = rp_pool.tile([128, 3, TT], f32, tag="m1")
                    z_all = rp_pool.tile([128, 3, TT], f32, tag="z")
                    m2p_all = rp_pool.tile([128, 3, TT], f32, tag="m2p")

                    specs = [
                        (wq_s, 0, 128, 128, 0),    # q heads 0,1
                        (wq_s, 128, 256, 128, 1),  # q heads 2,3
                        (wkv_s, 0, 2 * HD, 64, 2),  # k (rows 0:64) + v (64:128)
                    ]
                    kv_ps = None
                    for wsrc, e0, e1, nrows, ri in specs:
                        ps = ps_mm.tile([128, TT], f32, tag="mm2k", name=f"pj{ri}")
                        for idb in range(NDB):
                            nc.tensor.matmul(
                                ps,
                                wsrc[:, idb, e0:e1],
                                htile[:, idb, :],
                                start=(idb == 0),
                                stop=(idb == NDB - 1),
                            )
                        # RoPE input products; m2p (swapped z) comes via DMA
                        nc.vector.tensor_mul(
                            m1_all[:nrows, ri, :], ps[:nrows], cos_s[:nrows, tsl]
                        )
                        nc.vector.tensor_mul(
                            z_all[:nrows, ri, :], ps[:nrows], sinp_s[:nrows, tsl]
                        )
                        if ri == 2:
                            kv_ps = ps

                    # partition swap of z (32-row block pairs 0<->1, 2<->3)
                    for c, lo in ((0, 32), (1, 0), (2, 96), (3, 64)):
                        nc.sync.dma_start(
                            out=m2p_all[c * 32 : c * 32 + 32, :, :],
                            in_=z_all[lo : lo + 32, :, :],
                        )
                    # rope adds on gpsimd (DVE is the busier engine)
                    nc.gpsimd.tensor_add(
                        qt_s[0][:, tsl], m1_all[:, 0, :], m2p_all[:, 0, :]
                    )
                    nc.gpsimd.tensor_add(
                        qt_s[1][:, tsl], m1_all[:, 1, :], m2p_all[:, 1, :]
                    )
                    nc.gpsimd.tensor_add(
                        kt_s[0:64, tsl], m1_all[0:64, 2, :], m2p_all[0:64, 2, :]
                    )
                    # replicate k rows so odd q-heads can matmul from
                    # partition base 64 (tile_position row packing)
                    nc.sync.dma_start(out=kt_s[64:128, tsl], in_=kt_s[0:64, tsl])

                    # V: [d, t] -> [t, d] via PE transpose (V sits at
                    # partitions 64:128 of kv_ps; stage to SBUF first)
                    v_sb = vs_pool.tile([128, TT], f32, tag="v_sb")
                    nc.scalar.copy(out=v_sb[64:128, :], in_=kv_ps[64:128, :])
                    for c4 in range(TT // 128):
                        vt_ps = ps_mm.tile([128, HD], f32, tag="mm2k", name="vt")
                        nc.tensor.transpose(
                            vt_ps,
                            v_sb[64:128, c4 * 128 : (c4 + 1) * 128],
                            ident[64:128, 64:128],
                        )
                        nc.vector.tensor_copy(
                            out=va_s[:, it * 4 + c4, 0:HD], in_=vt_ps
                        )

                def attn_scores(it):
                    """scores + softmax + A@V + normalize for query tile it.
                    Returns the normalized per-head activations on_t."""
                    b, iq = it // 4, it % 4
                    q0 = iq * QT
                    qsl = slice(b * S + q0, b * S + q0 + QT)
                    on_t = [
                        on_pool.tile([128, QT], bf16, tag=f"on{i}", name=f"on{i}")
                        for i in range(2)
                    ]
                    ou_all = nm_pool.tile([65, 4, QT], f32, tag="ou")
                    for rp in range(2):
                        qtile = qt_s[rp]
                        nkb = (q0 // KBLK + 4) if causal else (S // KBLK)
                        o_ps = [
                            ps_o.tile(
                                [65, QT], f32, tag=f"o{i}", name=f"o{i}", bufs=1
                            )
                            for i in range(2)
                        ]
                        for kb in range(nkb):
                            ksl = slice(
                                b * S + kb * KBLK, b * S + (kb + 1) * KBLK
                            )
                            j = kb - q0 // KBLK
                            # query-column truncation: straddle block j only
                            # touches queries f >= 128*j
                            c0 = 128 * j if (causal and j > 0) else 0
                            s_ps = ps_s.tile([128, 2, QT], f32, tag="s")
                            pt = pt_pool.tile([128, 2, QT], bf16, tag="pt")
                            for h in range(2):
                                hb = h * 64
                                nc.tensor.matmul(
                                    s_ps[:, h, c0:QT],
                                    kt_s[hb : hb + 64, ksl],
                                    qtile[hb : hb + 64, qsl][:, c0:QT],
                                    start=True,
                                    stop=True,
                                )
                            if causal:
                                nc.scalar.activation(
                                    pt[:, :, c0:QT], s_ps[:, :, c0:QT],
                                    Exp, scale=0.125,
                                )
                            else:
                                mk = pt_pool.tile([128, QT], f32, tag="mk")
                                sm = pt_pool.tile([128, 2, QT], f32, tag="sm")
                                nc.sync.dma_start(
                                    out=mk,
                                    in_=maskT[
                                        kb * KBLK : (kb + 1) * KBLK,
                                        q0 : q0 + QT,
                                    ],
                                )
                                for h in range(2):
                                    nc.vector.scalar_tensor_tensor(
                                        out=sm[:, h, :],
                                        in0=s_ps[:, h, :],
                                        scalar=0.125,
                                        in1=mk,
                                        op0=mybir.AluOpType.mult,
                                        op1=mybir.AluOpType.add,
                                    )
                                nc.scalar.activation(pt, sm, Exp, scale=1.0)
                            for h in range(2):
                                if causal and j >= 0:
                                    nc.vector.tensor_mul(
                                        pt[:, h, c0:QT],
                                        pt[:, h, c0:QT],
                                        tri_s[:, j, c0:QT],
                                    )
                                nc.tensor.matmul(
                                    o_ps[h][:, c0:QT],
                                    va_s[:, b * (S // 128) + kb, :],
                                    pt[:, h, c0:QT],
                                    start=(kb == 0),
                                    stop=(kb == nkb - 1),
                                )
                        # evacuate PSUM promptly (frees o banks for rp=1)
                        for h in range(2):
                            nc.vector.tensor_copy(
                                out=ou_all[:, rp * 2 + h, :], in_=o_ps[h]
                            )

                    # batched normalization: the 4 denominator rows bounce
                    # through a [32, 64] layout so reciprocal uses 32 lanes
                    r32 = nm_pool.tile([32, 64], f32, tag="r32")
                    nc.sync.dma_start(out=r32, in_=ou_all[64:65, :, :])
                    r32r = nm_pool.tile([32, 64], f32, tag="r32r")
                    nc.vector.reciprocal(r32r, r32)
                    rec1 = nm_pool.tile([1, 4 * QT], f32, tag="rc")
                    nc.sync.dma_start(out=rec1, in_=r32r)
                    rec_b = nm_pool.tile([64, 4 * QT], f32, tag="rb")
                    nc.gpsimd.partition_broadcast(rec_b, rec1)
                    for rp in range(2):
                        # h=0 writes rows 0:64 (no partition shift -> gpsimd);
                        # h=1 writes rows 64:128 (shifted base -> DVE only)
                        hh = rp * 2
                        nc.gpsimd.tensor_mul(
                            on_t[rp][0:64, :],
                            ou_all[0:64, hh, :],
                            rec_b[:, hh * QT : (hh + 1) * QT],
                        )
                        nc.vector.tensor_mul(
                            on_t[rp][64:128, :],
                            ou_all[0:64, hh + 1, :],
                            rec_b[:, (hh + 1) * QT : (hh + 2) * QT],
                        )
                    return on_t

                def attn_out(it, on_t):
                    """Wo projection + coalesced bf16 output store."""
                    b, iq = it // 4, it % 4
                    q0 = iq * QT
                    qsl = slice(b * S + q0, b * S + q0 + QT)
                    out_acc = oa_pool.tile([128, D // 128, QT], bf16, tag="oacc")
                    for eb in range(D // 128):
                        wo_ps = ps_mm.tile([128, QT], f32, tag="mm2k", name="wo")
                        for db in range(2):
                            nc.tensor.matmul(
                                wo_ps,
                                wo_s[:, db, eb * 128 : (eb + 1) * 128],
                                on_t[db],
                                start=(db == 0),
                                stop=(db == 1),
                            )
                        # split PSUM evacuation across DVE and ACT
                        if eb % 2 == 0:
                            nc.vector.tensor_copy(
                                out=out_acc[:, eb, :], in_=wo_ps
                            )
                        else:
                            nc.scalar.copy(out=out_acc[:, eb, :], in_=wo_ps)
                    nc.sync.dma_start(out=outT_b3[:, :, qsl], in_=out_acc)

                # ---------------- the software-pipelined merged loop
                pending = None  # on_t of attention awaiting Wo projection
                for step in range(NT + 1):
                    if step >= 1:
                        on_t = attn_scores(step - 1)
                    if step <= NT - 1:
                        proj(step)
                    if step >= 1:
                        attn_out(step - 1, on_t)

    nc.compile()
    return nc


def _host_inputs(inputs, causal):
    """Shard + transpose the full inputs into 8 per-core input maps."""
    h = np.asarray(inputs["hidden_states"], np.float32)
    cos = np.asarray(inputs["position_cos"], np.float32)
    sin = np.asarray(inputs["position_sin"], np.float32)
    Wq = np.asarray(inputs["Wq"], np.float32)
    Wk = np.asarray(inputs["Wk"], np.float32)
    Wv = np.asarray(inputs["Wv"], np.float32)
    Wo = np.asarray(inputs["Wo"], np.float32)
    mask = np.asarray(inputs["attention_mask"], np.float32)[0, 0]

    hT = np.ascontiguousarray(h.reshape(T, D).T).astype(BF16)

    cosT = np.tile(cos.T, (1, B))                     # [64, T]
    sinT = np.tile(sin.T, (1, B))
    cos2 = np.ascontiguousarray(np.vstack([cosT, cosT]).astype(np.float32))
    s_signed = np.vstack([-sinT[0:32], sinT[32:64]])  # rot_half sign baked in
    sin2s = np.vstack([s_signed, s_signed])           # [128, T]
    # pre-swap so that z[p] = x[p]*sinp[p]; m2[p] = z[swap(p)] equals
    # rot_half(x)[p] * sin_signed[p]  (swap = 32-row block pairs 0<->1, 2<->3)
    swap_idx = np.concatenate(
        [np.arange(32, 64), np.arange(0, 32), np.arange(96, 128), np.arange(64, 96)]
    )
    sinp = np.ascontiguousarray(sin2s[swap_idx].astype(np.float32))

    maskT = np.ascontiguousarray(mask.T).astype(np.float32)

    in_maps = []
    for g in range(8):
        in_maps.append(
            {
                "hT": hT,
                "wqT": np.ascontiguousarray(
                    Wq[g * EQ : (g + 1) * EQ].T
                ).astype(BF16),
                "wkvT": np.ascontiguousarray(
                    np.concatenate(
                        [
                            Wk[g * HD : (g + 1) * HD].T,
                            Wv[g * HD : (g + 1) * HD].T,
                        ],
                        axis=1,
                    )
                ).astype(BF16),
                "woT": np.ascontiguousarray(
                    Wo[:, g * EQ : (g + 1) * EQ].T
                ).astype(BF16),
                "cos2": cos2,
                "sinp": sinp,
                "maskT": maskT,
            }
        )
    return in_maps


def _is_causal(mask):
    m = np.asarray(mask, np.float32)[0, 0]
    tri = np.tril(np.ones((S, S), bool))
    return bool(np.all(m[tri] == 0.0) and np.all(m[~tri] <= -1e8))


def _assemble(results):
    acc = np.zeros((D, T), np.float32)
    for r in results:
        acc += r["outT"].astype(np.float32)
    return np.ascontiguousarray(acc.reshape(D, B, S).transpose(1, 2, 0))


def kernel(**inputs) -> np.ndarray:
    from concourse.bass_utils import run_bass_kernel_spmd

    causal = _is_causal(inputs["attention_mask"])
    key = ("prog", causal)
    if key not in _CACHE:
        _CACHE[key] = _build_program(causal)
    nc = _CACHE[key]

    in_maps = _host_inputs(inputs, causal)
    res = run_bass_kernel_spmd(nc, in_maps, core_ids=list(range(8)))
    return _assemble(res.results)
